# revision 37
# baseline (speedup 1.0000x reference)
import os
import sys

import numpy as np

sys.path.insert(0, "/opt/trn_rl_repo")

from ml_dtypes import bfloat16

# ---- model constants (hardcoded from the problem spec) ----
N_ENTITY = 8000
N_REL = 100
N_GROUPS = 64
HIDDEN = 256
D_MODEL = 256
N_LAYERS = 2
N_HEADS = 4
GP = 64
GH = 2
D_TOTAL = D_MODEL * N_LAYERS          # 512
D_FEAT = D_TOTAL + 2 * HIDDEN         # 1024
B, L = 16, 512
Lh = L - 1                            # 511
N_CORES = 8
BS = B // N_CORES                     # 2 batches per core

TOKP = 512                            # padded token dim
NCH = 16                              # entity chunks of 512 (8000 -> 16*512 padded)
ECH = 512
KT_F = D_FEAT // 128                  # 8 k-tiles of the big matmul
NEG = -1.0e9

_NC_CACHE = {}
_LAST_RESULTS = None                  # BassKernelResults of the last run (for test.py)


def _build_module(has_intb: bool):
    import concourse.bass as bass
    import concourse.bacc as bacc
    import concourse.mybir as mybir
    import concourse.tile as tile
    from concourse.masks import make_identity

    f32 = mybir.dt.float32
    bf16 = mybir.dt.bfloat16
    AX = mybir.AxisListType.X
    OP = mybir.AluOpType
    AF = mybir.ActivationFunctionType

    nc = bacc.Bacc()

    # ---------------- DRAM parameters (per-core, host-packed) ----------------
    dp = nc.declare_dram_parameter
    xT_d = dp("xT", [128, BS, 2, TOKP], bf16, isOutput=False)
    curT_d = dp("curT", [128, BS, 2, TOKP], bf16, isOutput=False)
    wq_d = dp("wq", [128, N_LAYERS, 2, 256], bf16, isOutput=False)
    wk_d = dp("wk", [128, N_LAYERS, 2, 256], bf16, isOutput=False)
    wv_d = dp("wv", [128, N_LAYERS, 2, 256], bf16, isOutput=False)
    wo_d = dp("wo", [128, N_LAYERS, 2, 256], bf16, isOutput=False)
    cb_d = dp("cb", [128, 128], f32, isOutput=False)
    ohwT_d = dp("ohwT", [128, BS, 4, 64], bf16, isOutput=False)
    ohgT_d = dp("ohgT", [64, BS, TOKP], bf16, isOutput=False)
    gpwA_d = dp("gpwA", [128, 4, GP], bf16, isOutput=False)
    gpbias_d = dp("gpbias", [64, BS, GP], f32, isOutput=False)
    ginw_d = dp("ginw", [64, 3 * GP], bf16, isOutput=False)
    ginb_d = dp("ginb", [64, 3 * GP], f32, isOutput=False)
    goutw_d = dp("goutw", [64, GP], bf16, isOutput=False)
    goutb_d = dp("goutb", [64, GP], f32, isOutput=False)
    fw1_d = dp("fw1", [64, GP], bf16, isOutput=False)
    fb1_d = dp("fb1", [64, GP], f32, isOutput=False)
    fw2_d = dp("fw2", [64, GP], bf16, isOutput=False)
    fb2_d = dp("fb2", [64, GP], f32, isOutput=False)
    n1w_d = dp("n1w", [64, GP], f32, isOutput=False)
    n1b_d = dp("n1b", [64, GP], f32, isOutput=False)
    n2w_d = dp("n2w", [64, GP], f32, isOutput=False)
    n2b_d = dp("n2b", [64, GP], f32, isOutput=False)
    mgA_d = dp("mgA", [128, 4, D_FEAT], bf16, isOutput=False)
    mgG_d = dp("mgG", [64, D_FEAT], bf16, isOutput=False)
    midb_d = dp("midb", [128, BS, KT_F], f32, isOutput=False)
    intw_d = dp("intw", [128, NCH, KT_F, ECH], bf16, isOutput=False)
    if has_intb:
        intb_d = dp("intb", [1, NCH, ECH], bf16, isOutput=False)
    y_d = dp("y", [BS, Lh, N_ENTITY], f32, isOutput=True)

    with tile.TileContext(nc) as tc:
        import contextlib

        ctx = contextlib.ExitStack()
        with ctx:
            consts = ctx.enter_context(tc.tile_pool(name="consts", bufs=1))
            work = ctx.enter_context(tc.tile_pool(name="work", bufs=2))
            apool = ctx.enter_context(tc.tile_pool(name="apool", bufs=3))
            small = ctx.enter_context(tc.tile_pool(name="small", bufs=6))
            osb_pool = ctx.enter_context(tc.tile_pool(name="osb", bufs=3))
            intw_pool = ctx.enter_context(tc.tile_pool(name="intwp", bufs=3))
            ps_mm = ctx.enter_context(tc.tile_pool(name="ps_mm", bufs=2, space="PSUM"))
            ps_s = ctx.enter_context(tc.tile_pool(name="ps_s", bufs=2, space="PSUM"))
            ps_tp = ctx.enter_context(tc.tile_pool(name="ps_tp", bufs=2, space="PSUM"))
            ps_oi = ctx.enter_context(tc.tile_pool(name="ps_oi", bufs=2, space="PSUM"))

            def cload(dram, shape, dtype, name):
                t = consts.tile(shape, dtype, name=name, tag=name)
                nc.sync.dma_start(out=t[:], in_=dram[:])
                return t

            xT = cload(xT_d, [128, BS, 2, TOKP], bf16, "xT")
            curT = cload(curT_d, [128, BS, 2, TOKP], bf16, "curTt")
            wq = cload(wq_d, [128, N_LAYERS, 2, 256], bf16, "wq")
            wk = cload(wk_d, [128, N_LAYERS, 2, 256], bf16, "wk")
            wv = cload(wv_d, [128, N_LAYERS, 2, 256], bf16, "wv")
            wo = cload(wo_d, [128, N_LAYERS, 2, 256], bf16, "wo")
            cb = cload(cb_d, [128, 128], f32, "cb")
            ohwT = cload(ohwT_d, [128, BS, 4, 64], bf16, "ohwT")
            ohgT = cload(ohgT_d, [64, BS, TOKP], bf16, "ohgT")
            gpwA = cload(gpwA_d, [128, 4, GP], bf16, "gpwA")
            gpbias = cload(gpbias_d, [64, BS, GP], f32, "gpbias")
            ginw = cload(ginw_d, [64, 3 * GP], bf16, "ginw")
            ginb = cload(ginb_d, [64, 3 * GP], f32, "ginb")
            goutw = cload(goutw_d, [64, GP], bf16, "goutw")
            goutb = cload(goutb_d, [64, GP], f32, "goutb")
            fw1 = cload(fw1_d, [64, GP], bf16, "fw1")
            fb1 = cload(fb1_d, [64, GP], f32, "fb1")
            fw2 = cload(fw2_d, [64, GP], bf16, "fw2")
            fb2 = cload(fb2_d, [64, GP], f32, "fb2")
            n1w = cload(n1w_d, [64, GP], f32, "n1w")
            n1b = cload(n1b_d, [64, GP], f32, "n1b")
            n2w = cload(n2w_d, [64, GP], f32, "n2w")
            n2b = cload(n2b_d, [64, GP], f32, "n2b")
            mgA = cload(mgA_d, [128, 4, D_FEAT], bf16, "mgA")
            mgG = cload(mgG_d, [64, D_FEAT], bf16, "mgG")
            midb = cload(midb_d, [128, BS, KT_F], f32, "midb")
            if has_intb:
                intb = cload(intb_d, [1, NCH, ECH], bf16, "intb")
                ones_r = consts.tile([1, 128], bf16, name="ones_r", tag="ones_r")
                nc.vector.memset(ones_r[:], 1.0)

            ident = consts.tile([128, 128], bf16, name="ident", tag="ident")
            make_identity(nc, ident[:])
            eps_t = consts.tile([64, 1], f32, name="eps_t", tag="eps_t")
            nc.vector.memset(eps_t[:], 1e-5)
            one_t = consts.tile([128, 1], f32, name="one_t", tag="one_t")
            nc.vector.memset(one_t[:], 1.0)

            MM = nc.tensor.matmul

            def tchunks():
                for i in range(4):
                    t0 = i * 128
                    yield i, t0, min(128, Lh - t0)

            enh_all = []   # per batch: enhancedT SBUF tiles [128, 8, 512] bf16
            for b in range(BS):
                # ============ AttNHP core (transposed layout) ============
                enc = work.tile([128, 4, TOKP], bf16, name=f"enc{b}", tag="enc")
                cur_ap = [curT[:, b, 0, :], curT[:, b, 1, :]]
                for l in range(N_LAYERS):
                    qT = apool.tile([128, 2, TOKP], bf16, tag="qT", name=f"qT{b}{l}")
                    kT = apool.tile([128, 2, TOKP], bf16, tag="kT", name=f"kT{b}{l}")
                    for c in range(2):
                        m0 = c * 128
                        psq = ps_mm.tile([128, TOKP], f32, tag="mm", name="psq")
                        for kt in range(2):
                            MM(psq[:, :Lh], lhsT=wq[:, l, kt, m0:m0 + 128],
                               rhs=cur_ap[kt][:, :Lh], start=kt == 0, stop=kt == 1)
                        # (1/sqrt(hd) scale is folded into Wq host-side)
                        nc.vector.tensor_copy(qT[:, c, :Lh], psq[:, :Lh])
                        psk = ps_mm.tile([128, TOKP], f32, tag="mm", name="psk")
                        for kt in range(2):
                            MM(psk[:, :Lh], lhsT=wk[:, l, kt, m0:m0 + 128],
                               rhs=xT[:, b, kt, :Lh], start=kt == 0, stop=kt == 1)
                        nc.vector.tensor_copy(kT[:, c, :Lh], psk[:, :Lh])
                    v_sb = apool.tile([128, 4, 256], bf16, tag="v", name=f"v{b}{l}")
                    for i, t0, tw in tchunks():
                        psv = ps_mm.tile([128, 256], f32, tag="mm", name="psv")
                        for kt in range(2):
                            MM(psv[:tw, :], lhsT=xT[:, b, kt, t0:t0 + tw],
                               rhs=wv[:, l, kt, :], start=kt == 0, stop=kt == 1)
                        nc.vector.tensor_copy(v_sb[:tw, i, :], psv[:tw, :])

                    oT = apool.tile([128, 2, TOKP], bf16, tag="oT", name=f"oT{b}{l}")
                    for c in range(2):           # head pair
                        for hh in range(2):      # head within pair
                            h = 2 * c + hh
                            r0 = 64 * hh
                            for i, t0, qw in tchunks():
                                # causal trim: only keys k < valid can attend
                                valid = min(128 * (i + 1), Lh)
                                dw = valid - t0   # diagonal block width
                                s_ps = ps_s.tile([128, TOKP], f32, tag="s", name="s_ps")
                                MM(s_ps[:qw, :valid],
                                   lhsT=qT[r0:r0 + 64, c, t0:t0 + qw],
                                   rhs=kT[r0:r0 + 64, c, :valid],
                                   start=True, stop=True)
                                # mask only the diagonal block (upper triangle)
                                nc.vector.tensor_tensor(
                                    out=s_ps[:qw, t0:valid],
                                    in0=s_ps[:qw, t0:valid],
                                    in1=cb[:qw, :dw], op=OP.add)
                                # |s| is small (<~7): skip max-subtraction;
                                # masked entries are -1e9 -> exp underflows to 0
                                nm = small.tile([128, 2], f32, tag="nm", name="nm",
                                                bufs=12)
                                a_sb = apool.tile([128, TOKP], bf16, tag="a",
                                                  name="a_sb", bufs=6)
                                nc.scalar.activation(
                                    a_sb[:qw, :valid], s_ps[:qw, :valid], AF.Exp,
                                    accum_out=nm[:qw, 1:2])
                                rc = small.tile([128, 1], f32, tag="rc", name="rc",
                                                bufs=12)
                                nc.vector.reciprocal(rc[:qw], nm[:qw, 1:2])
                                nc.vector.tensor_scalar_mul(
                                    a_sb[:qw, :valid], a_sb[:qw, :valid], rc[:qw])
                                o_ps = ps_mm.tile([64, 128], f32, tag="mm",
                                                  name="o_ps")
                                for j in range(i + 1):   # causal: blocks k<=q only
                                    k0 = j * 128
                                    kw = min(128, Lh - k0)
                                    tp = ps_tp.tile([128, 128], bf16, tag="tp",
                                                    name="tp")
                                    nc.tensor.transpose(
                                        tp[:kw, :qw], a_sb[:qw, k0:k0 + kw],
                                        ident[:qw, :qw])
                                    aT = apool.tile([128, 128], bf16, tag="aT",
                                                    name="aT", bufs=8)
                                    nc.vector.tensor_copy(aT[:kw, :qw],
                                                          tp[:kw, :qw])
                                    MM(o_ps[:, :qw],
                                       lhsT=v_sb[:kw, j, 64 * h:64 * h + 64],
                                       rhs=aT[:kw, :qw],
                                       start=j == 0, stop=j == i)
                                nc.vector.tensor_copy(
                                    oT[r0:r0 + 64, c, t0:t0 + qw], o_ps[:, :qw])
                    for c in range(2):
                        m0 = c * 128
                        psp = ps_mm.tile([128, TOKP], f32, tag="mm", name="psp")
                        for kt in range(2):
                            MM(psp[:, :Lh], lhsT=wo[:, l, kt, m0:m0 + 128],
                               rhs=oT[:, kt, :Lh], start=kt == 0, stop=kt == 1)
                        nc.vector.tensor_tensor(
                            out=enc[:, 2 * l + c, :Lh], in0=psp[:, :Lh],
                            in1=cur_ap[c][:, :Lh], op=OP.add)
                    cur_ap = [enc[:, 2 * l, :], enc[:, 2 * l + 1, :]]

                # ============ group interaction ============
                # enc_gp[t, :] = enc_out[t, :] @ gp_w[:512]  (tokens on partitions)
                eg = work.tile([128, 4, GP], bf16, name=f"eg{b}", tag="eg")
                for i, t0, tw in tchunks():
                    pse = ps_mm.tile([128, GP], f32, tag="mm", name="pse")
                    for kt in range(4):
                        MM(pse[:tw, :], lhsT=enc[:, kt, t0:t0 + tw],
                           rhs=gpwA[:, kt, :], start=kt == 0, stop=kt == 3)
                    nc.vector.tensor_copy(eg[:tw, i, :], pse[:tw, :])
                # gp[g, :] = sum_t ohw[t, g] * enc_gp[t, :] + gp_bias
                psg = ps_mm.tile([64, GP], f32, tag="mm", name="psg")
                for i, t0, tw in tchunks():
                    MM(psg[:, :], lhsT=ohwT[:tw, b, i, :], rhs=eg[:tw, i, :],
                       start=i == 0, stop=i == 3)
                gp_sb = small.tile([64, GP], bf16, tag="gp", name="gp_sb")
                nc.vector.tensor_tensor(out=gp_sb[:], in0=psg[:],
                                        in1=gpbias[:, b, :], op=OP.add)

                def tr64(src_ap, w=GP, name="t64"):
                    # transpose a [64, w] bf16 SBUF tile -> [w, 64] bf16 SBUF
                    tp = ps_tp.tile([128, 64], bf16, tag="tp", name="tp64")
                    nc.tensor.transpose(tp[:w, :64], src_ap, ident[:64, :64])
                    out = small.tile([64 if w <= 64 else 128, 64], bf16,
                                     tag=name, name=name)
                    nc.scalar.copy(out[:w, :], tp[:w, :64])
                    return out

                gpT = tr64(gp_sb[:], name="gpT")
                psqkv = ps_mm.tile([64, 3 * GP], f32, tag="mm", name="psqkv")
                MM(psqkv[:], lhsT=gpT[:64, :], rhs=ginw[:], start=True, stop=True)
                qkv = small.tile([64, 3 * GP], bf16, tag="qkv", name="qkv")
                nc.vector.tensor_tensor(out=qkv[:], in0=psqkv[:], in1=ginb[:],
                                        op=OP.add)
                qT_g = tr64(qkv[:, 0:GP], name="qTg")
                kT_g = tr64(qkv[:, GP:2 * GP], name="kTg")
                pso = ps_mm.tile([64, GP], f32, tag="mm", name="pso")
                for h in range(GH):
                    hd0 = 32 * h
                    pss = ps_s.tile([64, 64], f32, tag="s", name="pss_g")
                    MM(pss[:], lhsT=qT_g[hd0:hd0 + 32, :], rhs=kT_g[hd0:hd0 + 32, :],
                       start=True, stop=True)
                    nmg = small.tile([64, 2], f32, tag="nmg", name="nmg")
                    ag = small.tile([64, 64], bf16, tag="ag", name="ag")
                    nc.scalar.activation(ag[:], pss[:], AF.Exp,
                                         scale=float(1.0 / np.sqrt(32.0)),
                                         accum_out=nmg[:, 1:2])
                    rcg = small.tile([64, 1], f32, tag="rcg", name="rcg")
                    nc.vector.reciprocal(rcg[:], nmg[:, 1:2])
                    nc.vector.tensor_scalar_mul(ag[:], ag[:], rcg[:])
                    agT = tr64(ag[:], w=64, name="agT")
                    MM(pso[:, hd0:hd0 + 32], lhsT=agT[:64, :],
                       rhs=qkv[:, 2 * GP + hd0:2 * GP + hd0 + 32],
                       start=True, stop=True)
                o_g = small.tile([64, GP], bf16, tag="og", name="o_g")
                nc.vector.tensor_copy(o_g[:], pso[:])
                oT_g = tr64(o_g[:], name="oTg")
                psga = ps_mm.tile([64, GP], f32, tag="mm", name="psga")
                MM(psga[:], lhsT=oT_g[:64, :], rhs=goutw[:], start=True, stop=True)
                h1 = small.tile([64, GP], f32, tag="h1", name="h1")
                nc.vector.tensor_tensor(out=h1[:], in0=psga[:], in1=goutb[:],
                                        op=OP.add)
                nc.vector.tensor_tensor(out=h1[:], in0=h1[:], in1=gp_sb[:],
                                        op=OP.add)

                def layernorm(dst, src, w_bc, b_bc, tag):
                    st = small.tile([64, 6], f32, tag=f"st{tag}", name=f"st{tag}")
                    nc.vector.bn_stats(out=st[:], in_=src[:])
                    mv = small.tile([64, 2], f32, tag=f"mv{tag}", name=f"mv{tag}")
                    nc.vector.bn_aggr(out=mv[:], in_=st[:])
                    sd = small.tile([64, 2], f32, tag=f"sd{tag}", name=f"sd{tag}")
                    # rstd = exp(-0.5 * ln(var + eps)); avoids Sqrt (separate
                    # ACT LUT set) and the reciprocal
                    nc.scalar.activation(sd[:, 0:1], mv[:, 1:2], AF.Ln,
                                         bias=eps_t[:, 0:1])
                    nc.scalar.activation(sd[:, 1:2], sd[:, 0:1], AF.Exp, scale=-0.5)
                    nc.vector.tensor_scalar_mul(mv[:, 0:1], mv[:, 0:1], -1.0)
                    nc.vector.tensor_scalar(out=dst[:], in0=src[:],
                                            scalar1=mv[:, 0:1], scalar2=sd[:, 1:2],
                                            op0=OP.add, op1=OP.mult)
                    nc.vector.tensor_tensor(out=dst[:], in0=dst[:], in1=w_bc[:],
                                            op=OP.mult)
                    nc.vector.tensor_tensor(out=dst[:], in0=dst[:], in1=b_bc[:],
                                            op=OP.add)

                gn = small.tile([64, GP], bf16, tag="gn", name="gn")
                layernorm(gn, h1, n1w, n1b, "1")
                gnT = tr64(gn[:], name="gnT")
                psf1 = ps_mm.tile([64, GP], f32, tag="mm", name="psf1")
                MM(psf1[:], lhsT=gnT[:64, :], rhs=fw1[:], start=True, stop=True)
                f1 = small.tile([64, GP], bf16, tag="f1", name="f1")
                nc.vector.tensor_tensor(out=psf1[:], in0=psf1[:], in1=fb1[:],
                                        op=OP.add)
                nc.scalar.activation(f1[:], psf1[:], AF.Relu)
                f1T = tr64(f1[:], name="f1T")
                psf2 = ps_mm.tile([64, GP], f32, tag="mm", name="psf2")
                MM(psf2[:], lhsT=f1T[:64, :], rhs=fw2[:], start=True, stop=True)
                h2 = small.tile([64, GP], f32, tag="h2", name="h2")
                nc.vector.tensor_tensor(out=h2[:], in0=psf2[:], in1=fb2[:],
                                        op=OP.add)
                nc.vector.tensor_tensor(out=h2[:], in0=h2[:], in1=gn[:], op=OP.add)
                gout = small.tile([64, GP], bf16, tag="gout", name="gout")
                layernorm(gout, h2, n2w, n2b, "2")

                # gatheredT[gp, t] = gout^T @ onehotG^T
                psgt = ps_s.tile([64, TOKP], f32, tag="s", name="psgt")
                MM(psgt[:, :Lh], lhsT=gout[:], rhs=ohgT[:, b, :Lh],
                   start=True, stop=True)
                gath = work.tile([64, TOKP], bf16, name=f"gath{b}", tag="gath")
                nc.scalar.copy(gath[:, :Lh], psgt[:, :Lh])

                # ============ merge -> enhancedT ============
                enh = work.tile([128, KT_F, TOKP], bf16, name=f"enh{b}", tag="enh")
                for cc in range(KT_F):
                    m0 = cc * 128
                    psm = ps_s.tile([128, TOKP], f32, tag="s", name="psm")
                    for kt in range(4):
                        MM(psm[:, :Lh], lhsT=mgA[:, kt, m0:m0 + 128],
                           rhs=enc[:, kt, :Lh], start=kt == 0, stop=False)
                    MM(psm[:, :Lh], lhsT=mgG[:, m0:m0 + 128], rhs=gath[:, :Lh],
                       start=False, stop=True)
                    nc.vector.tensor_scalar_add(enh[:, cc, :Lh], psm[:, :Lh],
                                                midb[:, b, cc:cc + 1])
                enh_all.append(enh)

            # ============ intensity head ============
            for ch in range(NCH):
                e0 = ch * ECH
                ew = min(ECH, N_ENTITY - e0)
                iw = intw_pool.tile([128, KT_F, ECH], bf16, tag="iw", name="iw")
                nc.sync.dma_start(out=iw[:], in_=intw_d[:, ch, :, :])
                for b in range(BS):
                    enh = enh_all[b]
                    for i, t0, tw in tchunks():
                        po = ps_oi.tile([128, ECH], f32, tag="oi", name="po")
                        for kt in range(KT_F):
                            MM(po[:tw, :], lhsT=enh[:, kt, t0:t0 + tw],
                               rhs=iw[:, kt, :],
                               start=kt == 0,
                               stop=(kt == KT_F - 1) and not has_intb)
                        if has_intb:
                            MM(po[:tw, :], lhsT=ones_r[0:1, :tw],
                               rhs=intb[0:1, ch, :], start=False, stop=True)
                        # softplus(x) = ln(exp(x) + 1)  (walrus has no softplus LUT)
                        et = osb_pool.tile([128, ECH], f32, tag="et", name="et")
                        nc.scalar.activation(et[:tw, :ew], po[:tw, :ew], AF.Exp)
                        ot = osb_pool.tile([128, ECH], f32, tag="osb", name="ot")
                        nc.scalar.activation(ot[:tw, :ew], et[:tw, :ew], AF.Ln,
                                             bias=one_t[:tw, 0:1])
                        nc.sync.dma_start(out=y_d[b, t0:t0 + tw, e0:e0 + ew],
                                          in_=ot[:tw, :ew])

    # All ACT funcs we use live in the 'natural_log_exp_and_others' LUT set,
    # but the table-load pass greedily picks the FIRST set containing each
    # func (exp -> exp_and_others, ln -> natural_log), ping-ponging the table
    # load hundreds of times. Blank every other set so the pass has exactly
    # one choice (set ids stay truthful for walrus).
    import concourse.bacc as bacc_mod
    from concourse import hw_specs
    orig_tables = hw_specs.get_activation_tables

    def _patched_tables(arch):
        t = orig_tables(arch)
        return {name: (s if name == "natural_log_exp_and_others" else set())
                for name, s in t.items()}

    bacc_mod.get_activation_tables = _patched_tables
    try:
        nc.compile()
    finally:
        bacc_mod.get_activation_tables = orig_tables
    return nc


def _get_nc(has_intb: bool):
    if has_intb not in _NC_CACHE:
        _NC_CACHE[has_intb] = _build_module(has_intb)
    return _NC_CACHE[has_intb]


def _time_enc(t):
    i = np.arange(D_MODEL // 2)
    freqs = np.exp(-np.log(10000.0) * (2.0 * i / D_MODEL)).astype(np.float32)
    ang = t[..., None].astype(np.float32) * freqs
    return np.concatenate([np.sin(ang), np.cos(ang)], axis=-1).astype(np.float32)


def _pack_T(a2d, pad_cols=TOKP):
    # [T, D] -> transposed, partition-major [128, D//128, pad_cols]
    T, D = a2d.shape
    out = np.zeros((128, D // 128, pad_cols), np.float32)
    aT = a2d.T.reshape(D // 128, 128, T).transpose(1, 0, 2)
    out[:, :, :T] = aT
    return out


def _prep_inputs(inputs):
    f32 = np.float32
    g = lambda k: np.asarray(inputs[k])
    subs, marks, objs = g("subs"), g("marks"), g("objs")
    times, dt = g("times").astype(f32), g("dt").astype(f32)
    mask, group_map = g("mask"), g("group_map")
    obj_embed = g("obj_embed").astype(f32)
    sub_embed = g("sub_embed").astype(f32)
    rel_embed = g("rel_embed").astype(f32)
    core_Wq = g("core_Wq").astype(f32)
    core_Wk = g("core_Wk").astype(f32)
    core_Wv = g("core_Wv").astype(f32)
    core_Wo = g("core_Wo").astype(f32)
    gp_w, gp_b = g("gp_w").astype(f32), g("gp_b").astype(f32)
    ga_in_w, ga_in_b = g("ga_in_w").astype(f32), g("ga_in_b").astype(f32)
    ga_out_w, ga_out_b = g("ga_out_w").astype(f32), g("ga_out_b").astype(f32)
    ffn_w1, ffn_b1 = g("ffn_w1").astype(f32), g("ffn_b1").astype(f32)
    ffn_w2, ffn_b2 = g("ffn_w2").astype(f32), g("ffn_b2").astype(f32)
    n1w, n1b = g("n1_w").astype(f32), g("n1_b").astype(f32)
    n2w, n2b = g("n2_w").astype(f32), g("n2_b").astype(f32)
    mg_w, mg_b = g("mg_w").astype(f32), g("mg_b").astype(f32)
    int_w, int_b = g("int_w").astype(f32), g("int_b").astype(f32)

    x = obj_embed[objs[:, :-1]] + _time_enc(times[:, :-1]) + _time_enc(dt[:, :-1])
    cur = _time_enc(times[:, 1:])                              # [B, Lh, 256]

    g_ids = group_map[subs * N_REL + marks][:, :-1]            # [B, Lh]
    fm = mask[:, :-1].astype(f32)                              # [B, Lh]
    sr = np.concatenate([sub_embed[subs[:, 0]], rel_embed[marks[:, 0]]], axis=-1)

    # host-folded biases
    mid_bias = sr @ mg_w[512:1024] + mg_b                      # [B, 1024]
    gp_low = sr @ gp_w[512:1024]                               # [B, 64]

    # diagonal-block causal bias [128, 128]: within a block, key j attends
    # iff j <= p (rows are queries, same pattern for every diagonal block)
    jj = np.arange(128)
    cb = np.where(jj[None, :] <= jj[:, None], 0.0, NEG).astype(f32)

    bf = lambda a: np.ascontiguousarray(a.astype(bfloat16))

    # shared (replicated) tensors
    def pack_w(W):  # [2, 256, 256] -> [128, 2, 2, 256]
        return bf(W.reshape(N_LAYERS, 2, 128, 256).transpose(2, 0, 1, 3))

    shared = {
        "wq": pack_w(core_Wq * np.float32(0.125)), "wk": pack_w(core_Wk),
        "wv": pack_w(core_Wv), "wo": pack_w(core_Wo),
        "cb": np.ascontiguousarray(cb),
        "gpwA": bf(gp_w[:512].reshape(4, 128, GP).transpose(1, 0, 2)),
        "ginw": bf(ga_in_w),
        "ginb": np.ascontiguousarray(np.broadcast_to(ga_in_b, (64, 3 * GP)).astype(f32)),
        "goutw": bf(ga_out_w),
        "goutb": np.ascontiguousarray(np.broadcast_to(ga_out_b, (64, GP)).astype(f32)),
        "fw1": bf(ffn_w1),
        "fb1": np.ascontiguousarray(np.broadcast_to(ffn_b1, (64, GP)).astype(f32)),
        "fw2": bf(ffn_w2),
        "fb2": np.ascontiguousarray(np.broadcast_to(ffn_b2, (64, GP)).astype(f32)),
        "n1w": np.ascontiguousarray(np.broadcast_to(n1w, (64, GP)).astype(f32)),
        "n1b": np.ascontiguousarray(np.broadcast_to(n1b, (64, GP)).astype(f32)),
        "n2w": np.ascontiguousarray(np.broadcast_to(n2w, (64, GP)).astype(f32)),
        "n2b": np.ascontiguousarray(np.broadcast_to(n2b, (64, GP)).astype(f32)),
        "mgA": bf(mg_w[:512].reshape(4, 128, D_FEAT).transpose(1, 0, 2)),
        "mgG": bf(mg_w[1024:1088]),
    }
    # int_w packed [128, 16, 8, 512]
    iwp = np.zeros((128, NCH, KT_F, ECH), f32)
    iw3 = int_w.reshape(KT_F, 128, N_ENTITY)                   # [kt, p, e]
    iwp[:, :15, :, :] = iw3[:, :, :15 * ECH].reshape(KT_F, 128, 15, ECH).transpose(1, 2, 0, 3)
    iwp[:, 15, :, :N_ENTITY - 15 * ECH] = iw3[:, :, 15 * ECH:].transpose(1, 0, 2)
    shared["intw"] = bf(iwp)

    has_intb = bool(np.any(int_b))
    if has_intb:
        ibp = np.zeros((1, NCH, ECH), f32)
        ibp[0, :15] = int_b[:15 * ECH].reshape(15, ECH)
        ibp[0, 15, :N_ENTITY - 15 * ECH] = int_b[15 * ECH:]
        shared["intb"] = bf(ibp)

    in_maps = []
    for c in range(N_CORES):
        bs = slice(BS * c, BS * (c + 1))
        xTc = np.zeros((128, BS, 2, TOKP), f32)
        curTc = np.zeros((128, BS, 2, TOKP), f32)
        ohwTc = np.zeros((128, BS, 4, 64), f32)
        ohgTc = np.zeros((64, BS, TOKP), f32)
        gpbias_c = np.zeros((64, BS, GP), f32)
        midb_c = np.zeros((128, BS, KT_F), f32)
        for bl, bg in enumerate(range(BS * c, BS * (c + 1))):
            xTc[:, bl] = _pack_T(x[bg])
            curTc[:, bl] = _pack_T(cur[bg])
            cnt = np.bincount(g_ids[bg], weights=fm[bg], minlength=N_GROUPS)[:N_GROUPS]
            w = fm[bg] / np.maximum(cnt, 1.0)[g_ids[bg]]       # [Lh]
            ohw = np.zeros((Lh, N_GROUPS), f32)
            ohw[np.arange(Lh), g_ids[bg]] = w
            # ohw is [Lh, 64]: pack tokens onto partitions: [128, 4, 64]
            tmp = np.zeros((4 * 128, N_GROUPS), f32)
            tmp[:Lh] = ohw
            ohwTc[:, bl] = tmp.reshape(4, 128, N_GROUPS).transpose(1, 0, 2)
            ohg = np.zeros((N_GROUPS, TOKP), f32)
            ohg[g_ids[bg], np.arange(Lh)] = fm[bg]
            ohgTc[:, bl] = ohg
            gpbias_c[:, bl] = gp_low[bg][None, :] * (cnt > 0)[:, None] + gp_b[None, :]
            midb_c[:, bl] = mid_bias[bg].reshape(KT_F, 128).T
        m = dict(shared)
        m["xT"] = bf(xTc)
        m["curT"] = bf(curTc)
        m["ohwT"] = bf(ohwTc)
        m["ohgT"] = bf(ohgTc)
        m["gpbias"] = np.ascontiguousarray(gpbias_c)
        m["midb"] = np.ascontiguousarray(midb_c)
        in_maps.append(m)
    return in_maps, has_intb


def kernel(**inputs):
    global _LAST_RESULTS
    in_maps, has_intb = _prep_inputs(inputs)
    nc = _get_nc(has_intb)
    from concourse.bass_utils import run_bass_kernel_spmd
    res = run_bass_kernel_spmd(nc, in_maps, core_ids=list(range(N_CORES)))
    _LAST_RESULTS = res
    out = np.concatenate([res.results[c]["y"] for c in range(N_CORES)], axis=0)
    return np.ascontiguousarray(out.astype(np.float32))
